# revision 20
# baseline (speedup 1.0000x reference)
"""Trainium2 Bass kernel for a 7-layer ternary-weight (BitNet) 1D conv
feature extractor with exact-erf GELU after each layer.

Contract: kernel(**inputs) takes the FULL inputs from setup_inputs()
(x: [8, 160000] f32, w0..w6 / b0..b6 conv params) and returns the full
output [8, 256, 500] f32.

Strategy: data-parallel over batch - one batch element per NeuronCore.
Weights are ternarized on host; the per-tensor absmean scale folds into
the GELU's per-partition scale operand, so the on-chip weights are bare
signs (exact in fp16 AND fp8).  Every conv is a chain of accumulating
matmuls into fp32 PSUM.

Performance structure:
 - L0 (contraction 10) runs 2x row-tiled: array rows 0:32 / 64:96
   process adjacent 512-position blocks concurrently (measured 2x).
 - Activations flowing into L1..L4 are stored fp8e4 (e4m3): CPU
   simulation puts the end-to-end rel-L2 at 6.8e-3 (budget 2e-2).
   This enables DoubleRow fp8 matmuls on the M=128 chains of L1-L4:
   tap pairs {t0,t1} contract 256 rows in one ~1.1x-cost matmul, and
   for the cin=192 layers the leftover-channel groups {tap0P, combP}
   form a second DoubleRow pair.  L5/L6 stay fp16 (fp8 there fails the
   error budget).
 - The 64-wide cout chunk (ch 128:192) of L1/L2/L3 is produced as
   column-tile pairs writing PSUM partitions 0:64 (even positions) /
   64:128 (odd), stored parity-split ("P-layout", col = pos//2 + 1).
   That halves those chunks' GELU-evacuation columns and removes the
   shifted-duplicate DMAs; consumers read the parity tile with the
   {tap0P, combP} groups (zero rows pad tap0P to 128 so no PE mode
   switches occur inside a chain).
 - Emission is software-pipelined over 4096-position L0 chunks with a
   >= 1-chunk consumer lag so no matmul blocks on a just-issued GELU;
   deep layers fill the PE while the scalar engine chews through the
   ACT-heavy early layers.
"""

import numpy as np

# (in_ch, out_ch, kernel, stride, pad) - fixed problem geometry
LAYERS = [(1, 128, 10, 5, 4), (128, 192, 3, 2, 1), (192, 192, 3, 2, 1),
          (192, 192, 3, 2, 1), (192, 256, 3, 2, 1), (256, 256, 4, 2, 1),
          (256, 256, 4, 2, 1)]
T_IN = 160000
LOUT = [32000, 16000, 8000, 4000, 2000, 1000, 500]
N_CORES = 8
NT = 512        # matmul free-dim tile (one fp32 PSUM bank)
USE_DR = False  # DoubleRow fp8 matmuls (toggle for bisection)
A0C = 4096      # L0-output chunk (ring buffered)
XTC = 4096      # L0 positions per x tile

# fp16 weight columns: L0, L5, L6
W16 = {0: 0, 5: 128, 6: 128 + 8 * 256}
TOT16 = 128 + 16 * 256
# fp8 weight columns: L1..L4.  Block layout per layer (cout = c):
#   [0, 2c)    "drstd": tap0 block [0,c) then tap1 block [c,2c), cin 0:128
#   [2c, 3c)   "t2":    tap 2 of cin 0:128
#   [3c, 5c)   "drpc":  tap0P block then combP block
#              (tap0P: rows 64:128 = tap0 of ch 128:192, rows 0:64 zero;
#               combP: rows 0:64 = tap1, rows 64:128 = tap2 of ch 128:192)
# L1 (cin=128) has only drstd + t2.  DoubleRow k-tile pairs are the two
# adjacent blocks (dim-1 stride = cout, ISA-legal for LDWEIGHTS).
W8 = {1: 0, 2: 576, 3: 576 + 960, 4: 576 + 2 * 960}
TOT8 = 576 + 2 * 960 + 1280


def _chunks(c):
    return [(0, min(c, 128))] + ([(128, c - 128)] if c > 128 else [])


def _bcols():
    nb = 0
    bcols = {}
    for i, (cin, cout, k, s, p) in enumerate(LAYERS):
        for mi, _ in enumerate(_chunks(cout)):
            bcols[(i, mi)] = nb
            nb += 2  # bias col + scale col
    return bcols, nb


def _pack_host(ws, bs):
    """Ternarize weights; pack fp16 (L0/L5/L6) + fp8 (L1-L4) signs and
    fp32 bias+scale."""
    import ml_dtypes
    F8 = ml_dtypes.float8_e4m3fn
    bcols, nb = _bcols()
    wpk = np.zeros((128, TOT16), np.float16)
    wpk8 = np.zeros((128, TOT8), np.float32)  # built f32, cast at end
    bpk = np.zeros((128, nb), np.float32)
    for i, (cin, cout, k, s, p) in enumerate(LAYERS):
        w = np.asarray(ws[i], np.float32)
        scale = max(float(np.mean(np.abs(w))), 1e-5)
        sign = np.clip(np.round(w / scale), -1.0, 1.0)  # [cout, cin, k]
        if i == 0:
            blk = sign[:, 0, :].T  # [10, 128]
            wpk[0:k, 0:128] = blk
            wpk[64:64 + k, 0:128] = blk
        elif i >= 5:
            base = W16[i]
            for gi, (ti, kk) in enumerate([(t, q) for t in range(2)
                                           for q in range(k)]):
                wpk[0:128, base + gi * cout:base + (gi + 1) * cout] = \
                    sign[:, 128 * ti:128 * ti + 128, kk].T
        else:
            b8 = W8[i]
            # drstd: tap0 block then tap1 block (cin 0:128)
            wpk8[0:128, b8:b8 + cout] = sign[:, 0:128, 0].T
            wpk8[0:128, b8 + cout:b8 + 2 * cout] = sign[:, 0:128, 1].T
            # t2
            wpk8[0:128, b8 + 2 * cout:b8 + 3 * cout] = sign[:, 0:128, 2].T
            if cin == 192:
                d = b8 + 3 * cout
                wpk8[64:128, d:d + cout] = sign[:, 128:192, 0].T
                wpk8[0:64, d + cout:d + 2 * cout] = sign[:, 128:192, 1].T
                wpk8[64:128, d + cout:d + 2 * cout] = sign[:, 128:192, 2].T
        b = np.asarray(bs[i], np.float32)
        for mi, (m0, msz) in enumerate(_chunks(cout)):
            c = bcols[(i, mi)]
            bpk[0:msz, c] = b[m0:m0 + msz]
            bpk[0:msz, c + 1] = scale
            if msz == 64:  # parity-packed chunk: duplicate for odd half
                bpk[64:128, c] = b[m0:m0 + msz]
                bpk[64:128, c + 1] = scale
    return wpk, wpk8.astype(F8), bpk


def _prep_x(xb):
    """Per-core L0 input, phase-major: xr[j, t] = xpad[5t + j],
    duplicated on rows 10:20 for the second L0 row-tile."""
    xpad = np.zeros(T_IN + 16, np.float16)
    xpad[4:4 + T_IN] = xb.astype(np.float16)
    L = LOUT[0]
    xr = np.empty((10, L), np.float16)
    for j in range(10):
        xr[j, :] = xpad[j:j + 5 * L:5]
    out = np.empty((20, L), np.float16)
    out[0:10] = xr
    out[10:20] = xr
    return out


_CACHE = {}


def _build():
    """Build + compile the Bass program (weight-data-independent)."""
    if "nc" in _CACHE:
        return _CACHE["nc"]
    from concourse import bacc
    import concourse.mybir as mybir
    import concourse.tile as tile
    from concourse.ap import AP

    F16 = mybir.dt.float16
    F32 = mybir.dt.float32
    F8 = mybir.dt.float8e4
    DR = mybir.MatmulPerfMode.DoubleRow
    GELU = mybir.ActivationFunctionType.Gelu
    bcols, nb = _bcols()

    nc = bacc.Bacc("TRN2")
    xr_d = nc.dram_tensor("xr", [20, LOUT[0]], F16, kind="ExternalInput")
    wp_d = nc.dram_tensor("wp", [128, TOT16], F16, kind="ExternalInput")
    w8_d = nc.dram_tensor("w8", [128, TOT8], F8, kind="ExternalInput")
    bp_d = nc.dram_tensor("bp", [128, nb], F32, kind="ExternalInput")
    y_d = nc.dram_tensor("y", [256, 500], F32, kind="ExternalOutput")

    def drpair(base2, m0):
        """View a [128, 2*blk] block-pair slice as [128, 2, 128] with
        the k-tile dim spanning the two blocks, m sliced at m0."""
        blk = base2.shape[1] // 2
        return base2.rearrange("p (j m) -> p j m", j=2)[:, :, m0:m0 + 128]

    def overlap2(base):
        """View [128, N] as [128, 2, N] with both dims stride 1 (cols
        n and n+1 form the DoubleRow k-tile pair)."""
        return AP(base.tensor, base.offset,
                  [list(base.ap[0]), [1, 2], list(base.ap[1])])

    with tile.TileContext(nc) as tc:
        pools = []

        def mkpool(name, bufs=1, space="SBUF"):
            p = tc.alloc_tile_pool(name=name, bufs=bufs, space=space)
            pools.append(p)
            return p

        wpool = mkpool("wpool")
        wt = wpool.tile([128, TOT16], F16, name="wt")
        w8 = wpool.tile([128, TOT8], F8, name="w8")
        bt = wpool.tile([128, nb], F32, name="bt")

        opool = mkpool("opool")
        stage = opool.tile([128, 1000], F32, name="stage")
        scratch = opool.tile([128, 512], F16, name="scratch")
        xpool = mkpool("xpool", bufs=3)
        a0pool = mkpool("a0pool", bufs=2)

        # PSUM: l0ps [128, 2048] (4 banks) single-buffered for L0
        # burst-pairs; mpool 2 x [128, 1024] rotating for everything else.
        l0pool = tc.alloc_tile_pool(name="l0pool", bufs=1, space="PSUM")
        mpool = tc.alloc_tile_pool(name="mpool", bufs=2, space="PSUM")

        # PE warm-up junk matmuls + GELU table preload while the first
        # input DMAs are in flight.
        nc.vector.memset(scratch[:, :], 0.0)

        def junk_mms(n):
            jp = mpool.tile([128, 1024], F32, name="ps", tag="ps")
            for _ in range(n):
                nc.tensor.matmul(jp[:, 0:512], scratch[:, 0:128],
                                 scratch[:, :], start=True, stop=True)

        junk_mms(8)
        nc.scalar.activation(scratch[0:128, 256:288], scratch[0:128, 0:32],
                             GELU)
        junk_mms(8)

        # fully-resident activation buffers.
        # std tiles: [128, lout+4], col = 1 + position, col 0 + tail
        # zero-padded.  parity tiles: [128, lout//2+2], partitions 0:64
        # even positions / 64:128 odd, col = 1 + position//2.
        # L0..L3 outputs are fp8e4; L4/L5 outputs fp16.
        std_t = {}
        par_t = {}
        for i in range(1, 6):
            cout = LAYERS[i][1]
            lout = LOUT[i]
            dt_i = F8 if i <= 3 else F16
            pool = mkpool(f"apool{i}")
            tiles = []
            for mi in range(1 if cout == 192 else 2):
                t = pool.tile([128, lout + 4], dt_i, name=f"a{i}_{mi}")
                nc.vector.memset(t[:, 0:1], 0.0)
                nc.vector.memset(t[:, lout + 1:lout + 3], 0.0)
                tiles.append(t)
            std_t[i] = tiles
            if cout == 192:
                t = pool.tile([128, lout // 2 + 2], dt_i, name=f"p{i}")
                nc.vector.memset(t[:, 0:1], 0.0)
                nc.vector.memset(t[:, lout // 2 + 1:lout // 2 + 2], 0.0)
                par_t[i] = t

        def emit_m0_unit(i, t0, nst, mi, srcs, dst):
            """One M=128 supertile of layer i, outputs [t0, t0+nst) of
            cout chunk mi.  L2-L4: fp8 DoubleRow chains; L5/L6: fp16."""
            cin, cout, k, s, p = LAYERS[i]
            m0 = 128 * mi
            stdp, parp = srcs
            ps = mpool.tile([128, 1024], F32, name="ps", tag="ps")
            for j0 in range(0, nst, NT):
                n = min(NT, nst - j0)
                tt = t0 + j0
                if i <= 4:
                    b8 = W8[i]
                    n_acc = 3 if cin == 192 else 2
                    if USE_DR:
                        # a=0: DoubleRow taps {0,1} of cin 0:128
                        lhsT = drpair(w8[0:128, b8:b8 + 2 * cout], m0)
                        rhs = stdp[0][0:128, 2 * tt:2 * tt + 2 * n] \
                            .rearrange("p (n j) -> p j n", j=2)
                        nc.tensor.matmul(ps[:, j0:j0 + n], lhsT, rhs,
                                         start=True, stop=False, perf_mode=DR)
                    else:
                        for j in (0, 1):
                            lhsT = w8[0:128, b8 + j * cout + m0:
                                      b8 + j * cout + m0 + 128]
                            rhs = stdp[0][0:128, 2 * tt + j:
                                          2 * tt + j + 2 * n - 1:2]
                            nc.tensor.matmul(ps[:, j0:j0 + n], lhsT, rhs,
                                             start=(j == 0), stop=False)
                    # tap 2 of cin 0:128
                    lhsT = w8[0:128, b8 + 2 * cout + m0:
                              b8 + 2 * cout + m0 + 128]
                    rhs = stdp[0][0:128, 2 * tt + 2:2 * tt + 2 + 2 * n - 1:2]
                    nc.tensor.matmul(ps[:, j0:j0 + n], lhsT, rhs,
                                     start=False, stop=(n_acc == 2))
                    if cin == 192:
                        d = b8 + 3 * cout
                        if USE_DR:
                            # DoubleRow {tap0P, combP} on the parity tile
                            lhsT = drpair(w8[0:128, d:d + 2 * cout], m0)
                            rhs = overlap2(parp[0:128, tt:tt + n])
                            nc.tensor.matmul(ps[:, j0:j0 + n], lhsT, rhs,
                                             start=False, stop=True,
                                             perf_mode=DR)
                        else:
                            for j in (0, 1):
                                lhsT = w8[0:128, d + j * cout + m0:
                                          d + j * cout + m0 + 128]
                                rhs = parp[0:128, tt + j:tt + j + n]
                                nc.tensor.matmul(ps[:, j0:j0 + n], lhsT, rhs,
                                                 start=False, stop=(j == 1))
                else:
                    base = W16[i]
                    for a, (ti, kk) in enumerate([(t, q) for t in range(2)
                                                  for q in range(k)]):
                        wb = base + a * cout + m0
                        rhs = stdp[ti][0:128, 2 * tt + kk:
                                       2 * tt + kk + 2 * n - 1:2]
                        nc.tensor.matmul(ps[:, j0:j0 + n],
                                         wt[0:128, wb:wb + 128], rhs,
                                         start=(a == 0), stop=(a == 2 * k - 1))
            bc = bcols[(i, mi)]
            nc.scalar.activation(dst, ps[0:128, 0:nst], GELU,
                                 bias=bt[0:128, bc:bc + 1],
                                 scale=bt[0:128, bc + 1:bc + 2])

        def emit_m1_pair(i, p0, npos, srcs):
            """Column-tiled production of the 64-ch chunk (ch 128:192)
            of layer i (2..3) for outputs [p0, p0+npos): even positions
            into PSUM partitions 0:64, odd into 64:128.  Plain fp8
            matmuls (DoubleRow + column tiling is illegal)."""
            cin, cout, k, s, p = LAYERS[i]
            b8 = W8[i]
            stdp, parp = srcs
            v0 = p0 // 2
            nv = npos // 2
            gl = [("std", j) for j in range(3)] + [("pc", 0), ("pc", 1)]
            n_acc = len(gl)
            ps = mpool.tile([128, 1024], F32, name="ps", tag="ps")
            for a, (kind, j) in enumerate(gl):
                if kind == "std":
                    lhsT = w8[0:128, b8 + j * cout + 128:b8 + j * cout + 192]
                else:
                    d = b8 + 3 * cout
                    lhsT = w8[0:128, d + j * cout + 128:d + j * cout + 192]
                for j0 in range(0, nv, NT):
                    n = min(NT, nv - j0)
                    vv = v0 + j0
                    for hi in (0, 1):
                        if kind == "std":
                            c0 = 4 * vv + j + 2 * hi
                            rhs = stdp[0][0:128, c0:c0 + 4 * n - 3:4]
                        else:
                            c0 = 2 * vv + j + hi
                            rhs = parp[0:128, c0:c0 + 2 * n - 1:2]
                        nc.tensor.matmul(ps[64 * hi:64 * hi + 64, j0:j0 + n],
                                         lhsT, rhs, skip_group_check=True,
                                         start=(a == 0), stop=(a == n_acc - 1))
            bc = bcols[(i, 1)]
            nc.scalar.activation(par_t[i][0:128, 1 + v0:1 + v0 + nv],
                                 ps[0:128, 0:nv], GELU,
                                 bias=bt[0:128, bc:bc + 1],
                                 scale=bt[0:128, bc + 1:bc + 2])

        # ---- per-chunk L1 units ----
        n_ch = (LOUT[0] + A0C - 1) // A0C
        a0_tiles = [None] * n_ch

        def l1_units(c):
            """L1 emitter thunks for a0 chunk c."""
            cbase = c * A0C
            csz = min(A0C, LOUT[0] - cbase)
            lo, hi = cbase // 2, (cbase + csz) // 2
            units = []
            for t0 in range(lo, hi, 1024):
                nst = min(1024, hi - t0)

                def u(t0=t0, nst=nst, c=c, cbase=cbase):
                    # L1 m0 supertile: DR{t0,t1} + t2.  a0 col = pos-cbase+1
                    at = a0_tiles[c]
                    b8 = W8[1]
                    ps = mpool.tile([128, 1024], F32, name="ps", tag="ps")
                    for j0 in range(0, nst, NT):
                        n = min(NT, nst - j0)
                        tt = t0 + j0
                        if USE_DR:
                            lhsT = drpair(w8[0:128, b8:b8 + 2 * 192], 0)
                            rhs = at[0:128, 2 * tt - cbase:
                                     2 * tt - cbase + 2 * n] \
                                .rearrange("p (n j) -> p j n", j=2)
                            nc.tensor.matmul(ps[:, j0:j0 + n], lhsT, rhs,
                                             start=True, stop=False,
                                             perf_mode=DR)
                        else:
                            for j in (0, 1):
                                lhsT = w8[0:128, b8 + j * 192:
                                          b8 + j * 192 + 128]
                                cj = 2 * tt + j - cbase
                                rhs = at[0:128, cj:cj + 2 * n - 1:2]
                                nc.tensor.matmul(ps[:, j0:j0 + n], lhsT, rhs,
                                                 start=(j == 0), stop=False)
                        lhsT = w8[0:128, b8 + 384:b8 + 384 + 128]
                        c2 = 2 * tt + 2 - cbase
                        rhs = at[0:128, c2:c2 + 2 * n - 1:2]
                        nc.tensor.matmul(ps[:, j0:j0 + n], lhsT, rhs,
                                         start=False, stop=True)
                    bc = bcols[(1, 0)]
                    nc.scalar.activation(
                        std_t[1][0][0:128, 1 + t0:1 + t0 + nst],
                        ps[0:128, 0:nst], GELU,
                        bias=bt[0:128, bc:bc + 1],
                        scale=bt[0:128, bc + 1:bc + 2])
                units.append(u)
            for p0 in range(lo, hi, 2048):
                npos = min(2048, hi - p0)

                def u(p0=p0, npos=npos, c=c, cbase=cbase):
                    # L1 m1 column-tile pair: plain fp8 taps 0..2 at M=64
                    at = a0_tiles[c]
                    b8 = W8[1]
                    v0 = p0 // 2
                    nv = npos // 2
                    ps = mpool.tile([128, 1024], F32, name="ps", tag="ps")
                    for a in range(3):
                        if a < 2:
                            lhsT = w8[0:128, b8 + a * 192 + 128:
                                      b8 + a * 192 + 192]
                        else:
                            lhsT = w8[0:128, b8 + 384 + 128:b8 + 384 + 192]
                        for j0 in range(0, nv, NT):
                            n = min(NT, nv - j0)
                            vv = v0 + j0
                            for hi2 in (0, 1):
                                c0 = 4 * vv + a + 2 * hi2 - cbase
                                rhs = at[0:128, c0:c0 + 4 * n - 3:4]
                                nc.tensor.matmul(
                                    ps[64 * hi2:64 * hi2 + 64, j0:j0 + n],
                                    lhsT, rhs, skip_group_check=True,
                                    start=(a == 0), stop=(a == 2))
                    bc = bcols[(1, 1)]
                    nc.scalar.activation(par_t[1][0:128, 1 + v0:1 + v0 + nv],
                                         ps[0:128, 0:nv], GELU,
                                         bias=bt[0:128, bc:bc + 1],
                                         scale=bt[0:128, bc + 1:bc + 2])
                units.append(u)
            return units

        def deep_unit_list(i):
            """Ordered (need, end, thunk) units for layer i (2..6);
            need = highest input position the unit reads."""
            units = []
            lout = LOUT[i]
            cout = LAYERS[i][1]
            if i == 4:
                srcs = (std_t[3], par_t[3])
            elif i >= 5:
                srcs = (std_t[i - 1], None)
            else:
                srcs = (std_t[i - 1], par_t[i - 1])
            for s0 in range(0, lout, 1024):
                nst = min(1024, lout - s0)
                e = s0 + nst
                need = min(2 * e, LOUT[i - 1]) - 1
                for mi in range(1 if cout == 192 else 2):
                    if i < 6:
                        dst = std_t[i][mi][0:128, 1 + s0:1 + s0 + nst]
                    else:
                        dst = stage[0:128, 500 * mi + s0:500 * mi + s0 + nst]
                    units.append((need, e, lambda i=i, s0=s0, nst=nst, mi=mi,
                                  srcs=srcs, dst=dst:
                                  emit_m0_unit(i, s0, nst, mi, srcs, dst)))
                if cout == 192 and ((s0 + nst) % 2048 == 0
                                    or s0 + nst == lout):
                    p0 = (s0 + nst - 1) // 2048 * 2048
                    npos = s0 + nst - p0
                    units.append((need, e, lambda i=i, p0=p0, npos=npos,
                                  srcs=srcs: emit_m1_pair(i, p0, npos, srcs)))
            return units

        deep_lists = {}
        deep_ptr = {}

        def extend_layer(i, avail):
            """Collect layer-i units whose inputs (<= avail) are ready."""
            if i not in deep_lists:
                deep_lists[i] = deep_unit_list(i)
                deep_ptr[i] = 0
            out = []
            lst = deep_lists[i]
            while deep_ptr[i] < len(lst) and lst[deep_ptr[i]][0] <= avail:
                out.append(lst[deep_ptr[i]][2])
                emitted[i] = lst[deep_ptr[i]][1]
                deep_ptr[i] += 1
            return out

        wrest = [0]

        def after_first_xt():
            # Bulk weight DMA via SWDGE (gpsimd) so it shares round-robin
            # with instead of queuing ahead of the x-chunk DMAs.
            if wrest[0] == 1:
                nc.gpsimd.dma_start(out=w8[:, 576:TOT8],
                                    in_=w8_d.ap()[:, 576:TOT8])
                nc.gpsimd.dma_start(out=wt[:, 128:TOT16],
                                    in_=wp_d.ap()[:, 128:TOT16])
            wrest[0] += 1

        xt_tiles = {}

        def fetch_x(xb):
            """Issue (or return the already-issued) x DMA for chunk xb."""
            if xb >= LOUT[0]:
                return None
            if xb not in xt_tiles:
                xn = min(XTC, LOUT[0] - xb)
                xt = xpool.tile([128, XTC], F16, tag="xt", name=f"xt_{xb}")
                nc.sync.dma_start(out=xt[0:10, 0:xn],
                                  in_=xr_d.ap()[0:10, xb:xb + xn])
                nc.sync.dma_start(out=xt[64:74, 0:xn],
                                  in_=xr_d.ap()[10:20, xb:xb + xn])
                xt_tiles[xb] = xt
            return xt_tiles[xb]

        # ---- main pipelined loop over a0 chunks ----
        # Consumers are emitted with a >= 1-chunk lag behind their
        # producers so no unit ever blocks on a just-issued ACT.
        emitted = {i: 0 for i in range(1, 7)}
        snaps = []
        deepq = []
        for c in range(n_ch):
            cbase = c * A0C
            csz = min(A0C, LOUT[0] - cbase)
            at = a0pool.tile([128, A0C + 3], F8, tag="a0", name=f"a0_{c}")
            a0_tiles[c] = at
            if c == 0:
                nc.vector.memset(at[:, 0:1], 0.0)
            else:
                nc.vector.tensor_copy(at[:, 0:1],
                                      a0_tiles[c - 1][:, A0C:A0C + 1])
            l1q = l1_units(c - 1) if c >= 1 else []
            avail = snaps[c - 2] if c >= 2 else {}
            for i in range(2, 7):
                deepq += extend_layer(i, avail.get(i - 1, 0))
            li = di = 0
            for xb in range(cbase, cbase + csz, XTC):
                xn = min(XTC, cbase + csz - xb)
                xt = fetch_x(xb)
                if xb == 0:
                    # L0 + L1 weights + biases after the first x chunk
                    nc.sync.dma_start(out=wt[:, 0:128],
                                      in_=wp_d.ap()[:, 0:128])
                    nc.sync.dma_start(out=w8[:, 0:576],
                                      in_=w8_d.ap()[:, 0:576])
                    nc.sync.dma_start(out=bt[:, :], in_=bp_d.ap())
                after_first_xt()
                # prefetch next chunk's x so the DMA latency hides
                fetch_x(xb + XTC)
                for t0 in range(xb, xb + xn, 2048):
                    nn = min(2048, xb + xn - t0)
                    ps = l0pool.tile([128, 2048], F32, name="l0ps", tag="l0ps")
                    for b in range(0, nn, 1024):
                        n1 = min(512, nn - b)
                        xc = t0 - xb + b
                        nc.tensor.matmul(
                            ps[:, b:b + n1], wt[0:10, 0:128],
                            xt[0:10, xc:xc + n1], start=True, stop=True)
                        if nn - b > 512:
                            n2 = min(512, nn - b - 512)
                            nc.tensor.matmul(
                                ps[:, b + 512:b + 512 + n2], wt[64:74, 0:128],
                                xt[64:74, xc + 512:xc + 512 + n2],
                                start=True, stop=True)
                    bc = bcols[(0, 0)]
                    nc.scalar.activation(
                        at[0:128, t0 - cbase + 1:t0 - cbase + 1 + nn],
                        ps[0:128, 0:nn], GELU,
                        bias=bt[0:128, bc:bc + 1],
                        scale=bt[0:128, bc + 1:bc + 2])
                    if li < len(l1q):
                        l1q[li]()
                        li += 1
                    for _ in range(2):
                        if di < len(deepq):
                            deepq[di]()
                            di += 1
                        elif c <= 2 and li >= len(l1q):
                            junk_mms(3)
            while li < len(l1q):
                l1q[li]()
                li += 1
            if c >= 1:
                emitted[1] += min(A0C, LOUT[0] - (c - 1) * A0C) // 2
            deepq = deepq[di:]
            snaps.append(dict(emitted))

        # ---- tail: remaining work in dependency order ----
        for u in l1_units(n_ch - 1):
            u()
        for u in deepq:
            u()
        for i in range(2, 7):
            for u in extend_layer(i, LOUT[i - 1]):
                u()

        nc.sync.dma_start(out=y_d.ap()[0:128, :], in_=stage[:, 0:500])
        nc.sync.dma_start(out=y_d.ap()[128:256, :], in_=stage[:, 500:1000])
        mpool.release()
        l0pool.release()
        for p in reversed(pools):
            p.release()

    nc.compile()
    _CACHE["nc"] = nc
    return nc


def kernel(x, w0, b0, w1, b1, w2, b2, w3, b3, w4, b4, w5, b5, w6, b6):
    import os
    from concourse.bass_utils import run_bass_kernel_spmd

    ws = [w0, w1, w2, w3, w4, w5, w6]
    bs = [b0, b1, b2, b3, b4, b5, b6]
    wpk, wpk8, bpk = _pack_host(ws, bs)
    x = np.asarray(x, np.float32)
    in_maps = [{"xr": _prep_x(x[b]), "wp": wpk, "w8": wpk8, "bp": bpk}
               for b in range(N_CORES)]
    nc = _build()
    trace = bool(os.environ.get("BITCONV_TRACE"))
    res = run_bass_kernel_spmd(nc, in_maps, core_ids=list(range(N_CORES)),
                               trace=trace)
    if trace:
        print(f"HW exec time: {res.exec_time_ns} ns")
        _CACHE["last_results"] = res
    return np.stack([res.results[b]["y"] for b in range(N_CORES)], axis=0)


# revision 21
# speedup vs baseline: 1.1565x; 1.1565x over previous
"""Trainium2 Bass kernel for a 7-layer ternary-weight (BitNet) 1D conv
feature extractor with exact-erf GELU after each layer.

Contract: kernel(**inputs) takes the FULL inputs from setup_inputs()
(x: [8, 160000] f32, w0..w6 / b0..b6 conv params) and returns the full
output [8, 256, 500] f32.

Strategy: data-parallel over batch - one batch element per NeuronCore.
Weights are ternarized on host; the per-tensor absmean scale folds into
the GELU's per-partition scale operand, so the on-chip weights are bare
signs (exact in fp16 AND fp8).  Every conv is a chain of accumulating
matmuls into fp32 PSUM.

Performance structure:
 - L0 (contraction 10) runs 2x row-tiled: array rows 0:32 / 64:96
   process adjacent 512-position blocks concurrently (measured 2x).
 - Activations flowing into L1..L4 are stored fp8e4 (e4m3): CPU
   simulation puts the end-to-end rel-L2 at 6.8e-3 (budget 2e-2).
   This enables DoubleRow fp8 matmuls on the M=128 chains of L1-L4:
   tap pairs {t0,t1} contract 256 rows in one ~1.1x-cost matmul, and
   for the cin=192 layers the leftover-channel groups {tap0P, combP}
   form a second DoubleRow pair.  L5/L6 stay fp16 (fp8 there fails the
   error budget).
 - The 64-wide cout chunk (ch 128:192) of L1/L2/L3 is produced as
   column-tile pairs writing PSUM partitions 0:64 (even positions) /
   64:128 (odd), stored parity-split ("P-layout", col = pos//2 + 1).
   That halves those chunks' GELU-evacuation columns and removes the
   shifted-duplicate DMAs; consumers read the parity tile with the
   {tap0P, combP} groups (zero rows pad tap0P to 128 so no PE mode
   switches occur inside a chain).
 - Emission is software-pipelined over 4096-position L0 chunks with a
   >= 1-chunk consumer lag so no matmul blocks on a just-issued GELU;
   deep layers fill the PE while the scalar engine chews through the
   ACT-heavy early layers.
"""

import numpy as np

# (in_ch, out_ch, kernel, stride, pad) - fixed problem geometry
LAYERS = [(1, 128, 10, 5, 4), (128, 192, 3, 2, 1), (192, 192, 3, 2, 1),
          (192, 192, 3, 2, 1), (192, 256, 3, 2, 1), (256, 256, 4, 2, 1),
          (256, 256, 4, 2, 1)]
T_IN = 160000
LOUT = [32000, 16000, 8000, 4000, 2000, 1000, 500]
N_CORES = 8
NT = 512        # matmul free-dim tile (one fp32 PSUM bank)
USE_DR = False  # DoubleRow fp8 matmuls (toggle for bisection)
A0C = 4096      # L0-output chunk (ring buffered)
XTC = 4096      # L0 positions per x tile

# fp16 weight columns: L0, L5, L6
W16 = {0: 0, 5: 128, 6: 128 + 8 * 256}
TOT16 = 128 + 16 * 256
# fp8 weight columns: L1..L4.  Block layout per layer (cout = c):
#   [0, 2c)    "drstd": tap0 block [0,c) then tap1 block [c,2c), cin 0:128
#   [2c, 3c)   "t2":    tap 2 of cin 0:128
#   [3c, 5c)   "drpc":  tap0P block then combP block
#              (tap0P: rows 64:128 = tap0 of ch 128:192, rows 0:64 zero;
#               combP: rows 0:64 = tap1, rows 64:128 = tap2 of ch 128:192)
# L1 (cin=128) has only drstd + t2.  DoubleRow k-tile pairs are the two
# adjacent blocks (dim-1 stride = cout, ISA-legal for LDWEIGHTS).
W8 = {1: 0, 2: 576, 3: 576 + 960, 4: 576 + 2 * 960}
TOT8 = 576 + 2 * 960 + 1280


def _chunks(c):
    return [(0, min(c, 128))] + ([(128, c - 128)] if c > 128 else [])


def _bcols():
    nb = 0
    bcols = {}
    for i, (cin, cout, k, s, p) in enumerate(LAYERS):
        for mi, _ in enumerate(_chunks(cout)):
            bcols[(i, mi)] = nb
            nb += 2  # bias col + scale col
    return bcols, nb


def _pack_host(ws, bs):
    """Ternarize weights; pack fp16 (L0/L5/L6) + fp8 (L1-L4) signs and
    fp32 bias+scale."""
    import ml_dtypes
    F8 = ml_dtypes.float8_e4m3fn
    bcols, nb = _bcols()
    wpk = np.zeros((128, TOT16), np.float16)
    wpk8 = np.zeros((128, TOT8), np.float32)  # built f32, cast at end
    bpk = np.zeros((128, nb), np.float32)
    for i, (cin, cout, k, s, p) in enumerate(LAYERS):
        w = np.asarray(ws[i], np.float32)
        scale = max(float(np.mean(np.abs(w))), 1e-5)
        sign = np.clip(np.round(w / scale), -1.0, 1.0)  # [cout, cin, k]
        if i == 0:
            blk = sign[:, 0, :].T  # [10, 128]
            wpk[0:k, 0:128] = blk
            wpk[64:64 + k, 0:128] = blk
        elif i >= 5:
            base = W16[i]
            for gi, (ti, kk) in enumerate([(t, q) for t in range(2)
                                           for q in range(k)]):
                wpk[0:128, base + gi * cout:base + (gi + 1) * cout] = \
                    sign[:, 128 * ti:128 * ti + 128, kk].T
        else:
            b8 = W8[i]
            # drstd: tap0 block then tap1 block (cin 0:128)
            wpk8[0:128, b8:b8 + cout] = sign[:, 0:128, 0].T
            wpk8[0:128, b8 + cout:b8 + 2 * cout] = sign[:, 0:128, 1].T
            # t2
            wpk8[0:128, b8 + 2 * cout:b8 + 3 * cout] = sign[:, 0:128, 2].T
            if cin == 192:
                d = b8 + 3 * cout
                wpk8[64:128, d:d + cout] = sign[:, 128:192, 0].T
                wpk8[0:64, d + cout:d + 2 * cout] = sign[:, 128:192, 1].T
                wpk8[64:128, d + cout:d + 2 * cout] = sign[:, 128:192, 2].T
        b = np.asarray(bs[i], np.float32)
        for mi, (m0, msz) in enumerate(_chunks(cout)):
            c = bcols[(i, mi)]
            bpk[0:msz, c] = b[m0:m0 + msz]
            bpk[0:msz, c + 1] = scale
            if msz == 64:  # parity-packed chunk: duplicate for odd half
                bpk[64:128, c] = b[m0:m0 + msz]
                bpk[64:128, c + 1] = scale
    return wpk, wpk8.astype(F8), bpk


def _prep_x(xb):
    """Per-core L0 input, phase-major: xr[j, t] = xpad[5t + j],
    duplicated on rows 10:20 for the second L0 row-tile."""
    xpad = np.zeros(T_IN + 16, np.float16)
    xpad[4:4 + T_IN] = xb.astype(np.float16)
    L = LOUT[0]
    xr = np.empty((10, L), np.float16)
    for j in range(10):
        xr[j, :] = xpad[j:j + 5 * L:5]
    out = np.empty((20, L), np.float16)
    out[0:10] = xr
    out[10:20] = xr
    return out


_CACHE = {}


def _build():
    """Build + compile the Bass program (weight-data-independent)."""
    if "nc" in _CACHE:
        return _CACHE["nc"]
    from concourse import bacc
    import concourse.mybir as mybir
    import concourse.tile as tile
    from concourse.ap import AP

    F16 = mybir.dt.float16
    F32 = mybir.dt.float32
    F8 = mybir.dt.float8e4
    DR = mybir.MatmulPerfMode.DoubleRow
    GELU = mybir.ActivationFunctionType.Gelu
    bcols, nb = _bcols()

    nc = bacc.Bacc("TRN2")
    xr_d = nc.dram_tensor("xr", [20, LOUT[0]], F16, kind="ExternalInput")
    wp_d = nc.dram_tensor("wp", [128, TOT16], F16, kind="ExternalInput")
    w8_d = nc.dram_tensor("w8", [128, TOT8], F8, kind="ExternalInput")
    bp_d = nc.dram_tensor("bp", [128, nb], F32, kind="ExternalInput")
    y_d = nc.dram_tensor("y", [256, 500], F32, kind="ExternalOutput")

    def drpair(base2, m0):
        """View a [128, 2*blk] block-pair slice as [128, 2, 128] with
        the k-tile dim spanning the two blocks, m sliced at m0."""
        blk = base2.shape[1] // 2
        return base2.rearrange("p (j m) -> p j m", j=2)[:, :, m0:m0 + 128]

    def overlap2(base):
        """View [128, N] as [128, 2, N] with both dims stride 1 (cols
        n and n+1 form the DoubleRow k-tile pair)."""
        return AP(base.tensor, base.offset,
                  [list(base.ap[0]), [1, 2], list(base.ap[1])])

    with tile.TileContext(nc) as tc:
        pools = []

        def mkpool(name, bufs=1, space="SBUF"):
            p = tc.alloc_tile_pool(name=name, bufs=bufs, space=space)
            pools.append(p)
            return p

        wpool = mkpool("wpool")
        wt = wpool.tile([128, TOT16], F16, name="wt")
        w8 = wpool.tile([128, TOT8], F8, name="w8")
        bt = wpool.tile([128, nb], F32, name="bt")

        opool = mkpool("opool")
        stage = opool.tile([128, 1000], F32, name="stage")
        scratch = opool.tile([128, 512], F16, name="scratch")
        xpool = mkpool("xpool", bufs=3)
        a0pool = mkpool("a0pool", bufs=2)

        # PSUM: l0ps [128, 2048] (4 banks) single-buffered for L0
        # burst-pairs; mpool 2 x [128, 1024] rotating for everything else.
        l0pool = tc.alloc_tile_pool(name="l0pool", bufs=1, space="PSUM")
        mpool = tc.alloc_tile_pool(name="mpool", bufs=2, space="PSUM")

        # PE warm-up junk matmuls + GELU table preload while the first
        # input DMAs are in flight.
        nc.vector.memset(scratch[:, :], 0.0)

        def junk_mms(n):
            jp = mpool.tile([128, 1024], F32, name="ps", tag="ps")
            for _ in range(n):
                nc.tensor.matmul(jp[:, 0:512], scratch[:, 0:128],
                                 scratch[:, :], start=True, stop=True)

        junk_mms(8)
        nc.scalar.activation(scratch[0:128, 256:288], scratch[0:128, 0:32],
                             GELU)
        junk_mms(8)

        # fully-resident activation buffers.
        # std tiles: [128, lout+4], col = 1 + position, col 0 + tail
        # zero-padded.  parity tiles: [128, lout//2+2], partitions 0:64
        # even positions / 64:128 odd, col = 1 + position//2.
        # L0..L3 outputs are fp8e4; L4/L5 outputs fp16.
        std_t = {}
        par_t = {}
        for i in range(1, 6):
            cout = LAYERS[i][1]
            lout = LOUT[i]
            dt_i = F8 if i <= 3 else F16
            pool = mkpool(f"apool{i}")
            tiles = []
            for mi in range(1 if cout == 192 else 2):
                t = pool.tile([128, lout + 4], dt_i, name=f"a{i}_{mi}")
                nc.vector.memset(t[:, 0:1], 0.0)
                nc.vector.memset(t[:, lout + 1:lout + 3], 0.0)
                tiles.append(t)
            std_t[i] = tiles
            if cout == 192:
                t = pool.tile([128, lout // 2 + 2], dt_i, name=f"p{i}")
                nc.vector.memset(t[:, 0:1], 0.0)
                nc.vector.memset(t[:, lout // 2 + 1:lout // 2 + 2], 0.0)
                par_t[i] = t

        def emit_m0_unit(i, t0, nst, mi, srcs, dst):
            """One M=128 supertile of layer i, outputs [t0, t0+nst) of
            cout chunk mi.  L2-L4: fp8 DoubleRow chains; L5/L6: fp16."""
            cin, cout, k, s, p = LAYERS[i]
            m0 = 128 * mi
            stdp, parp = srcs
            ps = mpool.tile([128, 1024], F32, name="ps", tag="ps")
            for j0 in range(0, nst, NT):
                n = min(NT, nst - j0)
                tt = t0 + j0
                if i <= 4:
                    b8 = W8[i]
                    n_acc = 3 if cin == 192 else 2
                    if USE_DR:
                        # a=0: DoubleRow taps {0,1} of cin 0:128
                        lhsT = drpair(w8[0:128, b8:b8 + 2 * cout], m0)
                        rhs = stdp[0][0:128, 2 * tt:2 * tt + 2 * n] \
                            .rearrange("p (n j) -> p j n", j=2)
                        nc.tensor.matmul(ps[:, j0:j0 + n], lhsT, rhs,
                                         start=True, stop=False, perf_mode=DR)
                    else:
                        for j in (0, 1):
                            lhsT = w8[0:128, b8 + j * cout + m0:
                                      b8 + j * cout + m0 + 128]
                            rhs = stdp[0][0:128, 2 * tt + j:
                                          2 * tt + j + 2 * n - 1:2]
                            nc.tensor.matmul(ps[:, j0:j0 + n], lhsT, rhs,
                                             start=(j == 0), stop=False)
                    # tap 2 of cin 0:128
                    lhsT = w8[0:128, b8 + 2 * cout + m0:
                              b8 + 2 * cout + m0 + 128]
                    rhs = stdp[0][0:128, 2 * tt + 2:2 * tt + 2 + 2 * n - 1:2]
                    nc.tensor.matmul(ps[:, j0:j0 + n], lhsT, rhs,
                                     start=False, stop=(n_acc == 2))
                    if cin == 192:
                        d = b8 + 3 * cout
                        if USE_DR:
                            # DoubleRow {tap0P, combP} on the parity tile
                            lhsT = drpair(w8[0:128, d:d + 2 * cout], m0)
                            rhs = overlap2(parp[0:128, tt:tt + n])
                            nc.tensor.matmul(ps[:, j0:j0 + n], lhsT, rhs,
                                             start=False, stop=True,
                                             perf_mode=DR)
                        else:
                            for j in (0, 1):
                                lhsT = w8[0:128, d + j * cout + m0:
                                          d + j * cout + m0 + 128]
                                rhs = parp[0:128, tt + j:tt + j + n]
                                nc.tensor.matmul(ps[:, j0:j0 + n], lhsT, rhs,
                                                 start=False, stop=(j == 1))
                else:
                    base = W16[i]
                    for a, (ti, kk) in enumerate([(t, q) for t in range(2)
                                                  for q in range(k)]):
                        wb = base + a * cout + m0
                        rhs = stdp[ti][0:128, 2 * tt + kk:
                                       2 * tt + kk + 2 * n - 1:2]
                        nc.tensor.matmul(ps[:, j0:j0 + n],
                                         wt[0:128, wb:wb + 128], rhs,
                                         start=(a == 0), stop=(a == 2 * k - 1))
            bc = bcols[(i, mi)]
            nc.scalar.activation(dst, ps[0:128, 0:nst], GELU,
                                 bias=bt[0:128, bc:bc + 1],
                                 scale=bt[0:128, bc + 1:bc + 2])

        def emit_m1_pair(i, p0, npos, srcs):
            """Column-tiled production of the 64-ch chunk (ch 128:192)
            of layer i (2..3) for outputs [p0, p0+npos): even positions
            into PSUM partitions 0:64, odd into 64:128.  Plain fp8
            matmuls (DoubleRow + column tiling is illegal)."""
            cin, cout, k, s, p = LAYERS[i]
            b8 = W8[i]
            stdp, parp = srcs
            v0 = p0 // 2
            nv = npos // 2
            gl = [("std", j) for j in range(3)] + [("pc", 0), ("pc", 1)]
            n_acc = len(gl)
            ps = mpool.tile([128, 1024], F32, name="ps", tag="ps")
            for a, (kind, j) in enumerate(gl):
                if kind == "std":
                    lhsT = w8[0:128, b8 + j * cout + 128:b8 + j * cout + 192]
                else:
                    d = b8 + 3 * cout
                    lhsT = w8[0:128, d + j * cout + 128:d + j * cout + 192]
                for j0 in range(0, nv, NT):
                    n = min(NT, nv - j0)
                    vv = v0 + j0
                    for hi in (0, 1):
                        if kind == "std":
                            c0 = 4 * vv + j + 2 * hi
                            rhs = stdp[0][0:128, c0:c0 + 4 * n - 3:4]
                        else:
                            c0 = 2 * vv + j + hi
                            rhs = parp[0:128, c0:c0 + 2 * n - 1:2]
                        nc.tensor.matmul(ps[64 * hi:64 * hi + 64, j0:j0 + n],
                                         lhsT, rhs, skip_group_check=True,
                                         start=(a == 0), stop=(a == n_acc - 1))
            bc = bcols[(i, 1)]
            nc.scalar.activation(par_t[i][0:128, 1 + v0:1 + v0 + nv],
                                 ps[0:128, 0:nv], GELU,
                                 bias=bt[0:128, bc:bc + 1],
                                 scale=bt[0:128, bc + 1:bc + 2])

        # ---- per-chunk L1 units ----
        n_ch = (LOUT[0] + A0C - 1) // A0C
        a0_tiles = [None] * n_ch

        def l1_units(c):
            """L1 emitter thunks for a0 chunk c."""
            cbase = c * A0C
            csz = min(A0C, LOUT[0] - cbase)
            lo, hi = cbase // 2, (cbase + csz) // 2
            units = []
            for t0 in range(lo, hi, 1024):
                nst = min(1024, hi - t0)

                def u(t0=t0, nst=nst, c=c, cbase=cbase):
                    # L1 m0 supertile: DR{t0,t1} + t2.  a0 col = pos-cbase+1
                    at = a0_tiles[c]
                    b8 = W8[1]
                    ps = mpool.tile([128, 1024], F32, name="ps", tag="ps")
                    for j0 in range(0, nst, NT):
                        n = min(NT, nst - j0)
                        tt = t0 + j0
                        if USE_DR:
                            lhsT = drpair(w8[0:128, b8:b8 + 2 * 192], 0)
                            rhs = at[0:128, 2 * tt - cbase:
                                     2 * tt - cbase + 2 * n] \
                                .rearrange("p (n j) -> p j n", j=2)
                            nc.tensor.matmul(ps[:, j0:j0 + n], lhsT, rhs,
                                             start=True, stop=False,
                                             perf_mode=DR)
                        else:
                            for j in (0, 1):
                                lhsT = w8[0:128, b8 + j * 192:
                                          b8 + j * 192 + 128]
                                cj = 2 * tt + j - cbase
                                rhs = at[0:128, cj:cj + 2 * n - 1:2]
                                nc.tensor.matmul(ps[:, j0:j0 + n], lhsT, rhs,
                                                 start=(j == 0), stop=False)
                        lhsT = w8[0:128, b8 + 384:b8 + 384 + 128]
                        c2 = 2 * tt + 2 - cbase
                        rhs = at[0:128, c2:c2 + 2 * n - 1:2]
                        nc.tensor.matmul(ps[:, j0:j0 + n], lhsT, rhs,
                                         start=False, stop=True)
                    bc = bcols[(1, 0)]
                    nc.scalar.activation(
                        std_t[1][0][0:128, 1 + t0:1 + t0 + nst],
                        ps[0:128, 0:nst], GELU,
                        bias=bt[0:128, bc:bc + 1],
                        scale=bt[0:128, bc + 1:bc + 2])
                units.append(u)
            for p0 in range(lo, hi, 2048):
                npos = min(2048, hi - p0)

                def u(p0=p0, npos=npos, c=c, cbase=cbase):
                    # L1 m1 column-tile pair: plain fp8 taps 0..2 at M=64
                    at = a0_tiles[c]
                    b8 = W8[1]
                    v0 = p0 // 2
                    nv = npos // 2
                    ps = mpool.tile([128, 1024], F32, name="ps", tag="ps")
                    for a in range(3):
                        if a < 2:
                            lhsT = w8[0:128, b8 + a * 192 + 128:
                                      b8 + a * 192 + 192]
                        else:
                            lhsT = w8[0:128, b8 + 384 + 128:b8 + 384 + 192]
                        for j0 in range(0, nv, NT):
                            n = min(NT, nv - j0)
                            vv = v0 + j0
                            for hi2 in (0, 1):
                                c0 = 4 * vv + a + 2 * hi2 - cbase
                                rhs = at[0:128, c0:c0 + 4 * n - 3:4]
                                nc.tensor.matmul(
                                    ps[64 * hi2:64 * hi2 + 64, j0:j0 + n],
                                    lhsT, rhs, skip_group_check=True,
                                    start=(a == 0), stop=(a == 2))
                    bc = bcols[(1, 1)]
                    nc.scalar.activation(par_t[1][0:128, 1 + v0:1 + v0 + nv],
                                         ps[0:128, 0:nv], GELU,
                                         bias=bt[0:128, bc:bc + 1],
                                         scale=bt[0:128, bc + 1:bc + 2])
                units.append(u)
            return units

        def deep_unit_list(i):
            """Ordered (need, end, thunk) units for layer i (2..6);
            need = highest input position the unit reads."""
            units = []
            lout = LOUT[i]
            cout = LAYERS[i][1]
            if i == 4:
                srcs = (std_t[3], par_t[3])
            elif i >= 5:
                srcs = (std_t[i - 1], None)
            else:
                srcs = (std_t[i - 1], par_t[i - 1])
            for s0 in range(0, lout, 1024):
                nst = min(1024, lout - s0)
                e = s0 + nst
                need = min(2 * e, LOUT[i - 1]) - 1
                for mi in range(1 if cout == 192 else 2):
                    if i < 6:
                        dst = std_t[i][mi][0:128, 1 + s0:1 + s0 + nst]
                    else:
                        dst = stage[0:128, 500 * mi + s0:500 * mi + s0 + nst]
                    units.append((need, e, lambda i=i, s0=s0, nst=nst, mi=mi,
                                  srcs=srcs, dst=dst:
                                  emit_m0_unit(i, s0, nst, mi, srcs, dst)))
                if cout == 192 and ((s0 + nst) % 2048 == 0
                                    or s0 + nst == lout):
                    p0 = (s0 + nst - 1) // 2048 * 2048
                    npos = s0 + nst - p0
                    units.append((need, e, lambda i=i, p0=p0, npos=npos,
                                  srcs=srcs: emit_m1_pair(i, p0, npos, srcs)))
            return units

        deep_lists = {}
        deep_ptr = {}

        def extend_layer(i, avail):
            """Collect layer-i units whose inputs (<= avail) are ready."""
            if i not in deep_lists:
                deep_lists[i] = deep_unit_list(i)
                deep_ptr[i] = 0
            out = []
            lst = deep_lists[i]
            while deep_ptr[i] < len(lst) and lst[deep_ptr[i]][0] <= avail:
                out.append(lst[deep_ptr[i]][2])
                emitted[i] = lst[deep_ptr[i]][1]
                deep_ptr[i] += 1
            return out

        wrest = [0]

        def after_first_xt():
            # Bulk weight DMA via SWDGE (gpsimd) so it shares round-robin
            # with instead of queuing ahead of the x-chunk DMAs.
            if wrest[0] == 1:
                nc.gpsimd.dma_start(out=w8[:, 576:TOT8],
                                    in_=w8_d.ap()[:, 576:TOT8])
                nc.gpsimd.dma_start(out=wt[:, 128:TOT16],
                                    in_=wp_d.ap()[:, 128:TOT16])
            wrest[0] += 1

        # ---- main pipelined loop over a0 chunks ----
        # Consumers are emitted with a >= 1-chunk lag behind their
        # producers so no unit ever blocks on a just-issued ACT.
        emitted = {i: 0 for i in range(1, 7)}
        snaps = []
        deepq = []
        for c in range(n_ch):
            cbase = c * A0C
            csz = min(A0C, LOUT[0] - cbase)
            at = a0pool.tile([128, A0C + 3], F8, tag="a0", name=f"a0_{c}")
            a0_tiles[c] = at
            if c == 0:
                nc.vector.memset(at[:, 0:1], 0.0)
            else:
                nc.vector.tensor_copy(at[:, 0:1],
                                      a0_tiles[c - 1][:, A0C:A0C + 1])
            l1q = l1_units(c - 1) if c >= 1 else []
            avail = snaps[c - 2] if c >= 2 else {}
            for i in range(2, 7):
                deepq += extend_layer(i, avail.get(i - 1, 0))
            li = di = 0
            for xb in range(cbase, cbase + csz, XTC):
                xn = min(XTC, cbase + csz - xb)
                xt = xpool.tile([128, XTC], F16, tag="xt", name=f"xt_{xb}")
                nc.sync.dma_start(out=xt[0:10, 0:xn],
                                  in_=xr_d.ap()[0:10, xb:xb + xn])
                nc.sync.dma_start(out=xt[64:74, 0:xn],
                                  in_=xr_d.ap()[10:20, xb:xb + xn])
                if xb == 0:
                    # L0 + L1 weights + biases after the first x chunk
                    nc.sync.dma_start(out=wt[:, 0:128],
                                      in_=wp_d.ap()[:, 0:128])
                    nc.sync.dma_start(out=w8[:, 0:576],
                                      in_=w8_d.ap()[:, 0:576])
                    nc.sync.dma_start(out=bt[:, :], in_=bp_d.ap())
                after_first_xt()
                for t0 in range(xb, xb + xn, 2048):
                    nn = min(2048, xb + xn - t0)
                    ps = l0pool.tile([128, 2048], F32, name="l0ps", tag="l0ps")
                    for b in range(0, nn, 1024):
                        n1 = min(512, nn - b)
                        xc = t0 - xb + b
                        nc.tensor.matmul(
                            ps[:, b:b + n1], wt[0:10, 0:128],
                            xt[0:10, xc:xc + n1], start=True, stop=True)
                        if nn - b > 512:
                            n2 = min(512, nn - b - 512)
                            nc.tensor.matmul(
                                ps[:, b + 512:b + 512 + n2], wt[64:74, 0:128],
                                xt[64:74, xc + 512:xc + 512 + n2],
                                start=True, stop=True)
                    bc = bcols[(0, 0)]
                    nc.scalar.activation(
                        at[0:128, t0 - cbase + 1:t0 - cbase + 1 + nn],
                        ps[0:128, 0:nn], GELU,
                        bias=bt[0:128, bc:bc + 1],
                        scale=bt[0:128, bc + 1:bc + 2])
                    if li < len(l1q):
                        l1q[li]()
                        li += 1
                    for _ in range(2):
                        if di < len(deepq):
                            deepq[di]()
                            di += 1
                        elif c <= 1 and li >= len(l1q):
                            junk_mms(2)
                            break
            while li < len(l1q):
                l1q[li]()
                li += 1
            if c >= 1:
                emitted[1] += min(A0C, LOUT[0] - (c - 1) * A0C) // 2
            deepq = deepq[di:]
            snaps.append(dict(emitted))

        # ---- tail: remaining work in dependency order ----
        for u in l1_units(n_ch - 1):
            u()
        for u in deepq:
            u()
        for i in range(2, 7):
            for u in extend_layer(i, LOUT[i - 1]):
                u()

        nc.sync.dma_start(out=y_d.ap()[0:128, :], in_=stage[:, 0:500])
        nc.sync.dma_start(out=y_d.ap()[128:256, :], in_=stage[:, 500:1000])
        mpool.release()
        l0pool.release()
        for p in reversed(pools):
            p.release()

    nc.compile()
    _CACHE["nc"] = nc
    return nc


def kernel(x, w0, b0, w1, b1, w2, b2, w3, b3, w4, b4, w5, b5, w6, b6):
    import os
    from concourse.bass_utils import run_bass_kernel_spmd

    ws = [w0, w1, w2, w3, w4, w5, w6]
    bs = [b0, b1, b2, b3, b4, b5, b6]
    wpk, wpk8, bpk = _pack_host(ws, bs)
    x = np.asarray(x, np.float32)
    in_maps = [{"xr": _prep_x(x[b]), "wp": wpk, "w8": wpk8, "bp": bpk}
               for b in range(N_CORES)]
    nc = _build()
    trace = bool(os.environ.get("BITCONV_TRACE"))
    res = run_bass_kernel_spmd(nc, in_maps, core_ids=list(range(N_CORES)),
                               trace=trace)
    if trace:
        print(f"HW exec time: {res.exec_time_ns} ns")
        _CACHE["last_results"] = res
    return np.stack([res.results[b]["y"] for b in range(N_CORES)], axis=0)
